# revision 11
# baseline (speedup 1.0000x reference)
"""Non-local block (B=4, C_in=256, C_int=128, C_out=256, N=T*H*W=4096) on 8
Trainium2 NeuronCores.

Sharding: data-parallel over batch (4 batches) x query-halves (2) = 8 cores.
Each core holds one batch's full x (for keys/values); the host rotates each
core's columns so its 2048 queries are always columns 0:2048 (attention is
permutation-invariant over keys). Per core: theta/phi/g projections, the
[2048q x 4096k] attention with softmax (keys on partitions), and the output
projection for its query half. Host gathers the 8 [256, 2048] slices.

Engine layout: PE does scores+y+projections; Act does the 64 [128,1024] exp
tiles (~1.0us each -- the pace-setter); DVE accumulates the softmax
denominator as elementwise adds over the exp tiles (d_acc += at), replacing
the ones-matmuls that burned ~27us of PE in the old version; one tiny
stationary-ones matmul per group broadcasts 1/d at output time.  Pool
(gpsimd) evacuates projection/gT/y PSUM.  Projections are interleaved INTO
the attention stream so exp starts as soon as the first x chunk lands
instead of after all projections.

PSUM budget (8 banks): scores ring 2x[128,1024]=4, y accumulator
1x[128,1024]=2, projection ring 2x[128,512]=2 (transposes write bitcast
slices of the projection tiles).
"""

import sys
import types

import numpy as np

import concourse.bacc as bacc
import concourse.mybir as mybir
import concourse.tile as tile
from concourse.bass_utils import run_bass_kernel_spmd


def _install_ntff_hook():
    """If tracing is requested (BASS_TRACE=1) under axon, bass_utils imports
    antenv.axon_hooks, which this image lacks; register the equivalent hook
    from trn_agent_boot so tracing works instead of crashing."""
    try:
        import antenv.axon_hooks  # noqa: F401
        return
    except ImportError:
        pass
    try:
        from trn_agent_boot.trn_boot import _ntff_profile_via_ctypes

        hook = _ntff_profile_via_ctypes("/opt/axon/libaxon_pjrt.so")
    except Exception:
        hook = None
    mod = types.ModuleType("antenv.axon_hooks")
    mod.get_axon_ntff_profile_hook = lambda: hook
    mod.set_axon_ntff_profile_hook = lambda h: None
    sys.modules["antenv.axon_hooks"] = mod


_install_ntff_hook()

F32 = mybir.dt.float32
F32R = mybir.dt.float32r
AF = mybir.ActivationFunctionType
OP = mybir.AluOpType

P = 128
CI = 256  # input channels (2 chunks of 128)
CINT = 128  # intermediate channels
CO = 256  # output channels (2 blocks of 128)
N = 4096  # key/value positions (32 blocks of 128)
Q = 2048  # queries per core
B, T, H, W = 4, 4, 32, 32
NKB = N // P  # 32 key blocks

# dtype used for matmul operands (fp32 data produced as float32r runs the PE
# at full rate for free dims >= 256; plain float32 runs at 1/4 rate)
MM_DT = F32R


def build():
    nc = bacc.Bacc(None, target_bir_lowering=False, debug=False)

    xb = nc.dram_tensor("xb", [CI, N], F32, kind="ExternalInput").ap()
    # all weights/constants packed host-side into one array -> one DMA; the
    # projection weights arrive PRE-TRANSPOSED (host numpy):
    # cols [0:256]=wtT, [256:512]=wpT, [512:768]=wgT, [768:1024]=woT,
    # [1024:1152]=identity(f32r), [1152:1280]=ones, [1280:1285]=biases
    cpak = nc.dram_tensor("cpak", [P, 1285], F32, kind="ExternalInput").ap()
    oq = nc.dram_tensor("oq", [CO, Q], F32, kind="ExternalOutput").ap()

    with tile.TileContext(nc) as tc:
        with (
            tc.tile_pool(name="consts", bufs=1) as consts,
            tc.tile_pool(name="big", bufs=1) as big,
            tc.tile_pool(name="tmp", bufs=6) as tmp,
        ):
            # ---- constants on the sync queue; x chunks spread across the
            # scalar/vector/gpsimd queues in need-order so the first
            # projection can start as early as possible ----
            cpak_sb = consts.tile([P, 1285], MM_DT, tag="cpak")
            nc.sync.dma_start(cpak_sb[:], cpak.bitcast(MM_DT))
            wtT = cpak_sb[:, 0:256].rearrange("p (o c) -> p o c", o=2)
            wpT = cpak_sb[:, 256:512].rearrange("p (o c) -> p o c", o=2)
            wgT = cpak_sb[:, 512:768].rearrange("p (o c) -> p o c", o=2)
            woT = cpak_sb[:, 768:1024].rearrange("p (o c) -> p o c", o=2)
            identity_r = cpak_sb[:, 1024:1152]
            ones_sb = cpak_sb[:, 1152:1280]
            bt_sb = cpak_sb[:, 1280:1281].bitcast(F32)
            bp_sb = cpak_sb[:, 1281:1282].bitcast(F32)
            bg_sb = cpak_sb[:, 1282:1283].bitcast(F32)
            bo_sb = cpak_sb[:, 1283:1285].bitcast(F32)

            x_sb = big.tile([P, 2, N], MM_DT, tag="x")
            xbr = xb.rearrange("(o p) n -> p o n", p=P).bitcast(MM_DT)
            for eng, sl in (
                (nc.scalar, slice(0, 512)),
                (nc.gpsimd, slice(512, 1024)),
                (nc.scalar, slice(1024, 2048)),
                (nc.gpsimd, slice(2048, 3072)),
                (nc.sync, slice(3072, 4096)),
            ):
                eng.dma_start(x_sb[:, :, sl], xbr[:, :, sl])

            # SBUF buffers shared across phases
            theta_sb = big.tile([P, Q], MM_DT, tag="theta")
            phi_sb = big.tile([P, N], MM_DT, tag="phi")
            g_sb = big.tile([P, N], MM_DT, tag="g")
            gT_sb = big.tile([P, N], MM_DT, tag="gT")  # kb-blocked transpose
            y_sb = big.tile([P, Q], MM_DT, tag="y")
            d_acc = big.tile([P, Q], MM_DT, tag="dacc")
            d_pool = big.tile([P, Q], MM_DT, tag="dpool")
            out_sb = big.tile([P, 2, Q], F32, tag="out")
            oqr = oq.rearrange("(o p) q -> p o q", p=P)

            with (
                tc.tile_pool(name="ps_proj", bufs=2, space="PSUM") as ps_proj,
                tc.tile_pool(name="ps_s2", bufs=2, space="PSUM") as ps_s,
                tc.tile_pool(name="ps_y", bufs=1, space="PSUM") as ps_y,
            ):
                # ---- projection emitters (interleaved into attention) ----
                # PSUM evacuations must run on DVE or Act (GPSIMD compute
                # and DMA cannot touch PSUM): use DVE, which also carries
                # the softmax-denominator accumulation.
                def proj(which, j, on_act=False):
                    wT, bias, dst = {
                        "t": (wtT, bt_sb, theta_sb),
                        "p": (wpT, bp_sb, phi_sb),
                        "g": (wgT, bg_sb, g_sb),
                    }[which]
                    sl = slice(j * 512, (j + 1) * 512)
                    pp = ps_proj.tile([P, 512], F32, tag="pp", name=f"pp{which}{j}")
                    nc.tensor.matmul(
                        pp[:], wT[:, 0, :], x_sb[:, 0, sl], start=True, stop=False
                    )
                    nc.tensor.matmul(
                        pp[:], wT[:, 1, :], x_sb[:, 1, sl], start=False, stop=True
                    )
                    if on_act:
                        # Act is idle until the first exp: use it for the
                        # prologue evacuations so the DVE stays free
                        nc.scalar.activation(
                            out=dst[:, sl], in_=pp[:], func=AF.Identity,
                            bias=bias,
                        )
                    else:
                        nc.vector.tensor_scalar(
                            out=dst[:, sl], in0=pp[:],
                            scalar1=bias, scalar2=None, op0=OP.add,
                        )

                def trans_g4(j, on_act=False):
                    # transpose g columns 512j..512j+512 (4 key blocks) into
                    # bitcast slices of one projection-ring PSUM tile, then
                    # one copy into gT
                    sl = slice(j * 512, (j + 1) * 512)
                    pg = ps_proj.tile([P, 512], F32, tag="pp", name=f"pgt{j}")
                    for k in range(4):
                        ksl = slice(j * 512 + k * P, j * 512 + (k + 1) * P)
                        nc.tensor.transpose(
                            pg[:, k * P : (k + 1) * P].bitcast(MM_DT),
                            g_sb[:, ksl], identity_r,
                        )
                    if on_act:
                        nc.scalar.activation(
                            out=gT_sb[:, sl], in_=pg[:].bitcast(MM_DT), func=AF.Copy
                        )
                    else:
                        nc.vector.tensor_copy(
                            out=gT_sb[:, sl], in_=pg[:].bitcast(MM_DT)
                        )

                # deferred projection pieces, consumed two per kb from inside
                # group 0's attention loop.  phi/g chunk j must precede
                # scores kb=4j; trans quad j must precede y kb=4j; theta
                # j2/j3 (group 1's queries) must precede group 1.  Quads are
                # emitted ~3 slots after their g chunk so the PE transpose
                # never waits on the Pool evacuation of g.
                work = [(proj, ("p", 1)), (proj, ("g", 1)), (proj, ("t", 2))]
                for j in range(2, 8):
                    work += [
                        (proj, ("p", j)), (proj, ("g", j)), (trans_g4, j - 1)
                    ]
                work += [(proj, ("t", 3)), (trans_g4, 7)]

                # prologue: the minimum needed for scores/y at kb=0..3,
                # with PSUM evacuations on the still-idle Act engine
                proj("t", 0, on_act=True)
                proj("p", 0, on_act=True)
                proj("g", 0, on_act=True)
                proj("t", 1, on_act=True)
                trans_g4(0, on_act=True)

                def attn_group(gi, q0, qw, pending_out=None, evac_on_act=False):
                    """Emit one query group's attention.  Returns a closure
                    emitting the group's output projection (invoked a few kb
                    into the NEXT group so it fills PE gaps)."""
                    qsl = slice(q0, q0 + qw)
                    nh = qw // 512
                    with nc.named_scope(f"attn{gi}"):
                        y_ps = ps_y.tile([P, qw], F32, tag=f"y{qw}", name=f"y_ps{gi}")

                        def scores(kb):
                            s_ps = ps_s.tile(
                                [P, qw], F32, tag=f"s{qw}", name=f"s{gi}_{kb}"
                            )
                            for h in range(nh):
                                nc.tensor.matmul(
                                    s_ps[:, h * 512 : (h + 1) * 512],
                                    phi_sb[:, kb * P : (kb + 1) * P],
                                    theta_sb[:, q0 + h * 512 : q0 + (h + 1) * 512],
                                    start=True, stop=True,
                                )
                            return s_ps

                        s_cur = scores(0)
                        for kb in range(NKB):
                            at = tmp.tile(
                                [P, qw], MM_DT, tag="attn", name=f"at{gi}_{kb}"
                            )
                            if evac_on_act and kb == NKB - 1 and nh > 1:
                                # last exp of the kernel: split per 512 so the
                                # tail's y matmuls and evacuation start sooner
                                for h in range(nh):
                                    hsl = slice(h * 512, (h + 1) * 512)
                                    nc.scalar.activation(
                                        out=at[:, hsl], in_=s_cur[:, hsl], func=AF.Exp
                                    )
                            else:
                                nc.scalar.activation(
                                    out=at[:], in_=s_cur[:], func=AF.Exp
                                )
                            if kb + 1 < NKB:
                                # feed the PE the next scores before y(kb) so
                                # it is not idle while Act runs exp(kb)
                                s_cur = scores(kb + 1)
                            # deferred projection pieces fill the PE's wait
                            # for exp(kb); all drained by kb ~12 of group 0
                            if gi == 0:
                                for _ in range(2):
                                    if work:
                                        fn, arg = work.pop(0)
                                        fn(*arg) if fn is proj else fn(arg)
                            first, last = kb == 0, kb == NKB - 1
                            for h in range(nh):
                                hsl = slice(h * 512, (h + 1) * 512)
                                nc.tensor.matmul(
                                    y_ps[:, hsl], gT_sb[:, kb * P : (kb + 1) * P],
                                    at[:, hsl], start=first, stop=last,
                                )
                            # softmax denominator: accumulate exp tiles
                            # elementwise, split across the two otherwise
                            # idle engines -- even kb on DVE, odd kb on Pool
                            # (Pool adds run at ~0.42 roofline, so one tile
                            # every other kb is exactly its pace)
                            dacc_eng = nc.vector if kb % 2 == 0 else nc.gpsimd
                            dacc_buf = d_acc if kb % 2 == 0 else d_pool
                            if kb < 2:
                                dacc_eng.tensor_copy(
                                    out=dacc_buf[:, qsl], in_=at[:]
                                )
                            else:
                                dacc_eng.tensor_tensor(
                                    out=dacc_buf[:, qsl], in0=dacc_buf[:, qsl],
                                    in1=at[:], op=OP.add,
                                )
                            if kb == 3 and pending_out is not None:
                                pending_out()
                        # evacuate y quickly so the next group can take the
                        # PSUM bank; on the last group Act is idle after its
                        # final exp, so evacuate there, else on Pool
                        for h in range(nh):
                            hsl = slice(h * 512, (h + 1) * 512)
                            qhsl = slice(q0 + h * 512, q0 + (h + 1) * 512)
                            if evac_on_act:
                                nc.scalar.activation(
                                    out=y_sb[:, qhsl], in_=y_ps[:, hsl], func=AF.Copy
                                )
                            else:
                                nc.vector.tensor_copy(
                                    out=y_sb[:, qhsl], in_=y_ps[:, hsl]
                                )

                    def emit_outproj():
                        with nc.named_scope(f"outp{gi}"):
                            # broadcast the denominator across partitions with
                            # one small stationary-ones matmul per 512
                            # queries; reciprocal immediately so the scores
                            # ring slot frees fast
                            d_bc = ps_s.tile(
                                [P, qw], F32, tag=f"s{qw}", name=f"dbc{gi}"
                            )
                            rd = tmp.tile([P, qw], F32, tag="rd", name=f"rd{gi}")
                            for h in range(nh):
                                hsl = slice(h * 512, (h + 1) * 512)
                                qhsl = slice(q0 + h * 512, q0 + (h + 1) * 512)
                                nc.tensor.matmul(
                                    d_bc[:, hsl], ones_sb, d_acc[:, qhsl],
                                    start=True, stop=False,
                                )
                                nc.tensor.matmul(
                                    d_bc[:, hsl], ones_sb, d_pool[:, qhsl],
                                    start=False, stop=True,
                                )
                                nc.vector.reciprocal_approx_fast(
                                    out=rd[:, hsl], in_=d_bc[:, hsl],
                                )
                            for blk in range(2):
                                for h in range(nh):
                                    hsl = slice(h * 512, (h + 1) * 512)
                                    qhsl = slice(q0 + h * 512, q0 + (h + 1) * 512)
                                    po = ps_s.tile(
                                        [P, 512], F32, tag=f"s{qw}",
                                        name=f"po{gi}{blk}{h}",
                                    )
                                    nc.tensor.matmul(
                                        po[:], woT[:, blk, :], y_sb[:, qhsl],
                                        start=True, stop=True,
                                    )
                                    # out = (po + b_out) * rd in one DVE pass.
                                    nc.vector.scalar_tensor_tensor(
                                        out=out_sb[:, blk, qhsl], in0=po[:],
                                        scalar=bo_sb[:, blk : blk + 1],
                                        in1=rd[:, hsl],
                                        op0=OP.add, op1=OP.mult,
                                    )
                                    nc.sync.dma_start(
                                        oqr[:, blk, qhsl], out_sb[:, blk, qhsl]
                                    )

                    return emit_outproj

                out0 = attn_group(0, 0, 1024)
                out1 = attn_group(1, 1024, 1024, pending_out=out0,
                                  evac_on_act=True)
                out1()

    nc.compile()
    return nc


IDENT = np.eye(P, dtype=np.float32)

_NC_CACHE = None
LAST_EXEC_TIME_NS = None
LAST_TRACE = None
LAST_RESULTS = None


def _get_nc():
    global _NC_CACHE
    if _NC_CACHE is None:
        _NC_CACHE = build()
    return _NC_CACHE


def kernel(**inputs):
    x = np.ascontiguousarray(np.asarray(inputs["x"], dtype=np.float32))
    assert x.shape == (B, CI, T, H, W), x.shape
    xf = x.reshape(B, CI, N)
    w = {
        k: np.ascontiguousarray(np.asarray(inputs[k], dtype=np.float32))
        for k in (
            "w_theta", "b_theta", "w_phi", "b_phi", "w_g", "b_g", "w_out", "b_out"
        )
    }

    def proj_t(wm):
        # [p, o*128+c] = wm[c, o*128+p]
        return wm.T.reshape(2, P, P).transpose(1, 0, 2).reshape(P, 2 * P)

    woT_h = w["w_out"].reshape(2, P, CINT).transpose(2, 0, 1).reshape(P, 2 * P)
    CPAK = np.ascontiguousarray(
        np.concatenate(
            [
                proj_t(w["w_theta"]), proj_t(w["w_phi"]), proj_t(w["w_g"]),
                woT_h, IDENT, np.ones((P, P), np.float32),
                np.stack(
                    [
                        w["b_theta"], w["b_phi"], w["b_g"],
                        w["b_out"][:P], w["b_out"][P:],
                    ],
                    axis=1,
                ),
            ],
            axis=1,
        )
    )
    in_maps = []
    for core in range(8):
        b, h = core // 2, core % 2
        if h == 0:
            xcore = xf[b]
        else:
            xcore = np.ascontiguousarray(
                np.concatenate([xf[b][:, Q:], xf[b][:, :Q]], axis=1)
            )
        in_maps.append(
            {"xb": xcore, "cpak": CPAK}
        )

    nc = _get_nc()
    res = run_bass_kernel_spmd(nc, in_maps, core_ids=list(range(8)))
    global LAST_EXEC_TIME_NS, LAST_TRACE, LAST_RESULTS
    LAST_EXEC_TIME_NS = res.exec_time_ns
    LAST_TRACE = res.instructions_and_trace[1] if res.instructions_and_trace else None
    LAST_RESULTS = res

    out = np.empty((B, CO, N), np.float32)
    for core in range(8):
        b, h = core // 2, core % 2
        out[b][:, h * Q : (h + 1) * Q] = res.results[core]["oq"]
    return out.reshape(B, CO, T, H, W)


# revision 15
# speedup vs baseline: 1.0171x; 1.0171x over previous
"""Non-local block (B=4, C_in=256, C_int=128, C_out=256, N=T*H*W=4096) on 8
Trainium2 NeuronCores.

Sharding: data-parallel over batch (4 batches) x query-halves (2) = 8 cores.
Each core holds one batch's full x (for keys/values); the host rotates each
core's columns so its 2048 queries are always columns 0:2048 (attention is
permutation-invariant over keys). Per core: theta/phi/g projections, the
[2048q x 4096k] attention with softmax (keys on partitions), and the output
projection for its query half. Host gathers the 8 [256, 2048] slices.

Engine layout: PE does scores+y+projections; Act does the 64 [128,1024] exp
tiles (~1.0us each -- the pace-setter); DVE accumulates the softmax
denominator as elementwise adds over the exp tiles (d_acc += at), replacing
the ones-matmuls that burned ~27us of PE in the old version; one tiny
stationary-ones matmul per group broadcasts 1/d at output time.  Pool
(gpsimd) evacuates projection/gT/y PSUM.  Projections are interleaved INTO
the attention stream so exp starts as soon as the first x chunk lands
instead of after all projections.

PSUM budget (8 banks): scores ring 2x[128,1024]=4, y accumulator
1x[128,1024]=2, projection ring 2x[128,512]=2 (transposes write bitcast
slices of the projection tiles).
"""

import sys
import types

import numpy as np

import concourse.bacc as bacc
import concourse.mybir as mybir
import concourse.tile as tile
from concourse.bass_utils import run_bass_kernel_spmd


def _install_ntff_hook():
    """If tracing is requested (BASS_TRACE=1) under axon, bass_utils imports
    antenv.axon_hooks, which this image lacks; register the equivalent hook
    from trn_agent_boot so tracing works instead of crashing."""
    try:
        import antenv.axon_hooks  # noqa: F401
        return
    except ImportError:
        pass
    try:
        from trn_agent_boot.trn_boot import _ntff_profile_via_ctypes

        hook = _ntff_profile_via_ctypes("/opt/axon/libaxon_pjrt.so")
    except Exception:
        hook = None
    mod = types.ModuleType("antenv.axon_hooks")
    mod.get_axon_ntff_profile_hook = lambda: hook
    mod.set_axon_ntff_profile_hook = lambda h: None
    sys.modules["antenv.axon_hooks"] = mod


_install_ntff_hook()

F32 = mybir.dt.float32
F32R = mybir.dt.float32r
AF = mybir.ActivationFunctionType
OP = mybir.AluOpType

P = 128
CI = 256  # input channels (2 chunks of 128)
CINT = 128  # intermediate channels
CO = 256  # output channels (2 blocks of 128)
N = 4096  # key/value positions (32 blocks of 128)
Q = 2048  # queries per core
B, T, H, W = 4, 4, 32, 32
NKB = N // P  # 32 key blocks

# dtype used for matmul operands (fp32 data produced as float32r runs the PE
# at full rate for free dims >= 256; plain float32 runs at 1/4 rate)
MM_DT = F32R


def build():
    nc = bacc.Bacc(None, target_bir_lowering=False, debug=False)

    xb = nc.dram_tensor("xb", [CI, N], F32, kind="ExternalInput").ap()
    # all weights/constants packed host-side into one array -> one DMA; the
    # projection weights arrive PRE-TRANSPOSED (host numpy):
    # cols [0:256]=wtT, [256:512]=wpT, [512:768]=wgT, [768:1024]=woT,
    # [1024:1152]=identity(f32r), [1152:1280]=ones, [1280:1285]=biases
    cpak = nc.dram_tensor("cpak", [P, 1285], F32, kind="ExternalInput").ap()
    oq = nc.dram_tensor("oq", [CO, Q], F32, kind="ExternalOutput").ap()

    with tile.TileContext(nc) as tc:
        with (
            tc.tile_pool(name="consts", bufs=1) as consts,
            tc.tile_pool(name="big", bufs=1) as big,
            tc.tile_pool(name="tmp", bufs=6) as tmp,
        ):
            # ---- constants on the sync queue; x chunks spread across the
            # scalar/vector/gpsimd queues in need-order so the first
            # projection can start as early as possible ----
            cpak_sb = consts.tile([P, 1285], MM_DT, tag="cpak")
            nc.sync.dma_start(cpak_sb[:], cpak.bitcast(MM_DT))
            wtT = cpak_sb[:, 0:256].rearrange("p (o c) -> p o c", o=2)
            wpT = cpak_sb[:, 256:512].rearrange("p (o c) -> p o c", o=2)
            wgT = cpak_sb[:, 512:768].rearrange("p (o c) -> p o c", o=2)
            woT = cpak_sb[:, 768:1024].rearrange("p (o c) -> p o c", o=2)
            identity_r = cpak_sb[:, 1024:1152]
            ones_sb = cpak_sb[:, 1152:1280]
            bt_sb = cpak_sb[:, 1280:1281].bitcast(F32)
            bp_sb = cpak_sb[:, 1281:1282].bitcast(F32)
            bg_sb = cpak_sb[:, 1282:1283].bitcast(F32)
            bo_sb = cpak_sb[:, 1283:1285].bitcast(F32)

            x_sb = big.tile([P, 2, N], MM_DT, tag="x")
            xbr = xb.rearrange("(o p) n -> p o n", p=P).bitcast(MM_DT)
            for eng, sl in (
                (nc.scalar, slice(0, 512)),
                (nc.gpsimd, slice(512, 1024)),
                (nc.scalar, slice(1024, 2048)),
                (nc.gpsimd, slice(2048, 3072)),
                (nc.sync, slice(3072, 4096)),
            ):
                eng.dma_start(x_sb[:, :, sl], xbr[:, :, sl])

            # SBUF buffers shared across phases
            theta_sb = big.tile([P, Q], MM_DT, tag="theta")
            phi_sb = big.tile([P, N], MM_DT, tag="phi")
            g_sb = big.tile([P, N], MM_DT, tag="g")
            gT_sb = big.tile([P, N], MM_DT, tag="gT")  # kb-blocked transpose
            y_sb = big.tile([P, Q], MM_DT, tag="y")
            d_acc = big.tile([P, Q], MM_DT, tag="dacc")
            out_sb = big.tile([P, 2, Q], F32, tag="out")
            oqr = oq.rearrange("(o p) q -> p o q", p=P)

            with (
                tc.tile_pool(name="ps_s2", bufs=2, space="PSUM") as ps_s,
                tc.tile_pool(name="ps_y", bufs=1, space="PSUM") as ps_y,
            ):
                def attn_group(gi, q0, qw, work=None, ps_proj=None, ps_d=None,
                               pending_out=None, evac_on_act=False):
                    """Emit one query group's attention.

                    gi=0: softmax denominator accumulated fully on the DVE
                    (d_acc += exp tile), with deferred projection pieces
                    (`work`) interleaved one per kb.  gi=1: the projection
                    PSUM ring is closed, freeing two banks for a PE-side
                    denominator accumulator (ps_d) -- odd kb go to the PE as
                    tiny ones-matmuls, even kb to the DVE; the partials are
                    merged by one extra accumulating matmul at the end.
                    Returns a closure emitting the group's output projection
                    (invoked a few kb into the NEXT group to fill PE gaps).
                    """
                    qsl = slice(q0, q0 + qw)
                    nh = qw // 512
                    d_ps = None
                    with nc.named_scope(f"attn{gi}"):
                        y_ps = ps_y.tile([P, qw], F32, tag=f"y{qw}", name=f"y_ps{gi}")
                        if ps_d is not None:
                            d_ps = ps_d.tile([P, qw], F32, tag="dps", name=f"d_ps{gi}")

                        def scores(kb):
                            s_ps = ps_s.tile(
                                [P, qw], F32, tag=f"s{qw}", name=f"s{gi}_{kb}"
                            )
                            for h in range(nh):
                                nc.tensor.matmul(
                                    s_ps[:, h * 512 : (h + 1) * 512],
                                    phi_sb[:, kb * P : (kb + 1) * P],
                                    theta_sb[:, q0 + h * 512 : q0 + (h + 1) * 512],
                                    start=True, stop=True,
                                )
                            return s_ps

                        s_cur = scores(0)
                        for kb in range(NKB):
                            at = tmp.tile(
                                [P, qw], MM_DT, tag="attn", name=f"at{gi}_{kb}"
                            )
                            if evac_on_act and kb == NKB - 1 and nh > 1:
                                # last exp of the kernel: split per 512 so the
                                # tail's y matmuls and evacuation start sooner
                                for h in range(nh):
                                    hsl = slice(h * 512, (h + 1) * 512)
                                    nc.scalar.activation(
                                        out=at[:, hsl], in_=s_cur[:, hsl], func=AF.Exp
                                    )
                            else:
                                nc.scalar.activation(
                                    out=at[:], in_=s_cur[:], func=AF.Exp
                                )
                            if kb + 1 < NKB:
                                # feed the PE the next scores before y(kb) so
                                # it is not idle while Act runs exp(kb)
                                s_cur = scores(kb + 1)
                            # one deferred projection piece per kb (group 0)
                            if work:
                                fn, arg = work.pop(0)
                                fn(*arg)
                            first, last = kb == 0, kb == NKB - 1
                            for h in range(nh):
                                hsl = slice(h * 512, (h + 1) * 512)
                                nc.tensor.matmul(
                                    y_ps[:, hsl], gT_sb[:, kb * P : (kb + 1) * P],
                                    at[:, hsl], start=first, stop=last,
                                )
                            # softmax denominator
                            if d_ps is not None and kb % 2 == 1:
                                for h in range(nh):
                                    hsl = slice(h * 512, (h + 1) * 512)
                                    nc.tensor.matmul(
                                        d_ps[:, hsl], ones_sb, at[:, hsl],
                                        start=kb == 1, stop=False,
                                    )
                            elif kb == 0:
                                nc.vector.tensor_copy(
                                    out=d_acc[:, qsl], in_=at[:]
                                )
                            else:
                                nc.vector.tensor_tensor(
                                    out=d_acc[:, qsl], in0=d_acc[:, qsl],
                                    in1=at[:], op=OP.add,
                                )
                            if kb == 3 and pending_out is not None:
                                pending_out()
                        if d_ps is not None:
                            # fold the DVE partial into the PE accumulator
                            for h in range(nh):
                                hsl = slice(h * 512, (h + 1) * 512)
                                nc.tensor.matmul(
                                    d_ps[:, hsl], ones_sb,
                                    d_acc[:, q0 + h * 512 : q0 + (h + 1) * 512],
                                    start=False, stop=True,
                                )
                        # evacuate y quickly so the next group can take the
                        # PSUM bank; on the last group Act is idle after its
                        # final exp
                        for h in range(nh):
                            hsl = slice(h * 512, (h + 1) * 512)
                            qhsl = slice(q0 + h * 512, q0 + (h + 1) * 512)
                            nc.scalar.activation(
                                out=y_sb[:, qhsl], in_=y_ps[:, hsl], func=AF.Copy
                            )

                    def emit_outproj():
                        with nc.named_scope(f"outp{gi}"):
                            rd = tmp.tile([P, qw], F32, tag="rd", name=f"rd{gi}")
                            if d_ps is None:
                                # broadcast the DVE denominator across
                                # partitions with a small ones-matmul
                                d_bc = ps_s.tile(
                                    [P, qw], F32, tag=f"s{qw}", name=f"dbc{gi}"
                                )
                                for h in range(nh):
                                    hsl = slice(h * 512, (h + 1) * 512)
                                    nc.tensor.matmul(
                                        d_bc[:, hsl], ones_sb,
                                        d_acc[:, q0 + h * 512 : q0 + (h + 1) * 512],
                                        start=True, stop=True,
                                    )
                                    nc.vector.reciprocal_approx_fast(
                                        out=rd[:, hsl], in_=d_bc[:, hsl],
                                    )
                            else:
                                for h in range(nh):
                                    hsl = slice(h * 512, (h + 1) * 512)
                                    nc.vector.reciprocal_approx_fast(
                                        out=rd[:, hsl], in_=d_ps[:, hsl],
                                    )
                            for blk in range(2):
                                for h in range(nh):
                                    hsl = slice(h * 512, (h + 1) * 512)
                                    qhsl = slice(q0 + h * 512, q0 + (h + 1) * 512)
                                    po = ps_s.tile(
                                        [P, 512], F32, tag=f"s{qw}",
                                        name=f"po{gi}{blk}{h}",
                                    )
                                    nc.tensor.matmul(
                                        po[:], woT[:, blk, :], y_sb[:, qhsl],
                                        start=True, stop=True,
                                    )
                                    # out = (po + b_out) * rd in one DVE pass.
                                    nc.vector.scalar_tensor_tensor(
                                        out=out_sb[:, blk, qhsl], in0=po[:],
                                        scalar=bo_sb[:, blk : blk + 1],
                                        in1=rd[:, hsl],
                                        op0=OP.add, op1=OP.mult,
                                    )
                                    nc.sync.dma_start(
                                        oqr[:, blk, qhsl], out_sb[:, blk, qhsl]
                                    )

                    return emit_outproj

                with tc.tile_pool(name="ps_proj", bufs=2, space="PSUM") as ps_proj:
                    # ---- projection emitters -------------------------------
                    # PSUM evacuations must run on DVE or Act (GPSIMD compute
                    # and DMA cannot touch PSUM).  Projections go to Act
                    # (which has slack vs the DVE d-accumulation), transpose
                    # quads to DVE.
                    def proj(which, j, on_act=True):
                        wT, bias, dst = {
                            "t": (wtT, bt_sb, theta_sb),
                            "p": (wpT, bp_sb, phi_sb),
                            "g": (wgT, bg_sb, g_sb),
                        }[which]
                        sl = slice(j * 512, (j + 1) * 512)
                        pp = ps_proj.tile(
                            [P, 512], F32, tag="pp", name=f"pp{which}{j}"
                        )
                        nc.tensor.matmul(
                            pp[:], wT[:, 0, :], x_sb[:, 0, sl],
                            start=True, stop=False,
                        )
                        nc.tensor.matmul(
                            pp[:], wT[:, 1, :], x_sb[:, 1, sl],
                            start=False, stop=True,
                        )
                        if on_act:
                            nc.scalar.activation(
                                out=dst[:, sl], in_=pp[:], func=AF.Identity,
                                bias=bias,
                            )
                        else:
                            nc.vector.tensor_scalar(
                                out=dst[:, sl], in0=pp[:],
                                scalar1=bias, scalar2=None, op0=OP.add,
                            )

                    def trans_g4(j, on_act=False):
                        # transpose g columns 512j..512j+512 (4 key blocks)
                        # into bitcast slices of one projection-ring PSUM
                        # tile, then one copy into gT
                        sl = slice(j * 512, (j + 1) * 512)
                        pg = ps_proj.tile([P, 512], F32, tag="pp", name=f"pgt{j}")
                        for k in range(4):
                            ksl = slice(j * 512 + k * P, j * 512 + (k + 1) * P)
                            nc.tensor.transpose(
                                pg[:, k * P : (k + 1) * P].bitcast(MM_DT),
                                g_sb[:, ksl], identity_r,
                            )
                        if on_act:
                            nc.scalar.activation(
                                out=gT_sb[:, sl], in_=pg[:].bitcast(MM_DT),
                                func=AF.Copy,
                            )
                        else:
                            nc.vector.tensor_copy(
                                out=gT_sb[:, sl], in_=pg[:].bitcast(MM_DT)
                            )

                    # deferred pieces, one per kb of group 0.  phi/g chunk j
                    # must precede scores kb=4j; trans quad j must precede y
                    # kb=4j; theta j2/j3 (group 1's queries) before group 1.
                    # Quads trail their g chunk by ~3 slots so the PE
                    # transpose never waits on the Act evacuation of g.
                    work = [
                        (proj, ("p", 1)), (proj, ("g", 1)), (proj, ("t", 2)),
                        (trans_g4, (1,)),
                    ]
                    for j in range(2, 8):
                        work += [
                            (proj, ("p", j)), (proj, ("g", j)),
                            (trans_g4, (j,)),
                        ]
                    work += [(proj, ("t", 3))]
                    # piece k is emitted at kb=k (one per kb, just before
                    # y(kb)): quad j sits at position 3j (for j>=2; quad1 at
                    # 3) <= its deadline kb=4j, phi j at 3j-2 <= 4j-1

                    # prologue: minimum for scores/y at kb=0..3, evacuated on
                    # the still-idle Act engine
                    proj("t", 0)
                    proj("p", 0)
                    proj("g", 0)
                    proj("t", 1)
                    trans_g4(0, on_act=True)

                    out0 = attn_group(0, 0, 1024, work=work, ps_proj=ps_proj)

                with tc.tile_pool(name="ps_d", bufs=1, space="PSUM") as ps_d:
                    out1 = attn_group(1, 1024, 1024, ps_d=ps_d,
                                      pending_out=out0, evac_on_act=True)
                    out1()

    nc.compile()
    return nc


IDENT = np.eye(P, dtype=np.float32)

_NC_CACHE = None
LAST_EXEC_TIME_NS = None
LAST_TRACE = None
LAST_RESULTS = None


def _get_nc():
    global _NC_CACHE
    if _NC_CACHE is None:
        _NC_CACHE = build()
    return _NC_CACHE


def kernel(**inputs):
    x = np.ascontiguousarray(np.asarray(inputs["x"], dtype=np.float32))
    assert x.shape == (B, CI, T, H, W), x.shape
    xf = x.reshape(B, CI, N)
    w = {
        k: np.ascontiguousarray(np.asarray(inputs[k], dtype=np.float32))
        for k in (
            "w_theta", "b_theta", "w_phi", "b_phi", "w_g", "b_g", "w_out", "b_out"
        )
    }

    def proj_t(wm):
        # [p, o*128+c] = wm[c, o*128+p]
        return wm.T.reshape(2, P, P).transpose(1, 0, 2).reshape(P, 2 * P)

    woT_h = w["w_out"].reshape(2, P, CINT).transpose(2, 0, 1).reshape(P, 2 * P)
    CPAK = np.ascontiguousarray(
        np.concatenate(
            [
                proj_t(w["w_theta"]), proj_t(w["w_phi"]), proj_t(w["w_g"]),
                woT_h, IDENT, np.ones((P, P), np.float32),
                np.stack(
                    [
                        w["b_theta"], w["b_phi"], w["b_g"],
                        w["b_out"][:P], w["b_out"][P:],
                    ],
                    axis=1,
                ),
            ],
            axis=1,
        )
    )
    in_maps = []
    for core in range(8):
        b, h = core // 2, core % 2
        if h == 0:
            xcore = xf[b]
        else:
            xcore = np.ascontiguousarray(
                np.concatenate([xf[b][:, Q:], xf[b][:, :Q]], axis=1)
            )
        in_maps.append(
            {"xb": xcore, "cpak": CPAK}
        )

    nc = _get_nc()
    res = run_bass_kernel_spmd(nc, in_maps, core_ids=list(range(8)))
    global LAST_EXEC_TIME_NS, LAST_TRACE, LAST_RESULTS
    LAST_EXEC_TIME_NS = res.exec_time_ns
    LAST_TRACE = res.instructions_and_trace[1] if res.instructions_and_trace else None
    LAST_RESULTS = res

    out = np.empty((B, CO, N), np.float32)
    for core in range(8):
        b, h = core // 2, core % 2
        out[b][:, h * Q : (h + 1) * Q] = res.results[core]["oq"]
    return out.reshape(B, CO, T, H, W)


# revision 19
# speedup vs baseline: 1.1502x; 1.1308x over previous
"""Non-local block (B=4, C_in=256, C_int=128, C_out=256, N=T*H*W=4096) on 8
Trainium2 NeuronCores.

Sharding: data-parallel over batch (4 batches) x query-halves (2) = 8 cores.
Each core holds one batch's full x (for keys/values); the host rotates each
core's columns so its 2048 queries are always columns 0:2048 (attention is
permutation-invariant over keys). Per core: theta/phi/g projections, the
[2048q x 4096k] attention with softmax (keys on partitions), and the output
projection for its query half. Host gathers the 8 [256, 2048] slices.

Engine layout: PE does scores+y+projections; Act does the 64 [128,1024] exp
tiles (~1.0us each -- the pace-setter); DVE accumulates the softmax
denominator as elementwise adds over the exp tiles (d_acc += at), replacing
the ones-matmuls that burned ~27us of PE in the old version; one tiny
stationary-ones matmul per group broadcasts 1/d at output time.  Pool
(gpsimd) evacuates projection/gT/y PSUM.  Projections are interleaved INTO
the attention stream so exp starts as soon as the first x chunk lands
instead of after all projections.

PSUM budget (8 banks): scores ring 2x[128,1024]=4, y accumulator
1x[128,1024]=2, projection ring 2x[128,512]=2 (transposes write bitcast
slices of the projection tiles).
"""

import sys
import types

import numpy as np

import concourse.bacc as bacc
import concourse.mybir as mybir
import concourse.tile as tile
from concourse.bass_utils import run_bass_kernel_spmd


def _install_ntff_hook():
    """If tracing is requested (BASS_TRACE=1) under axon, bass_utils imports
    antenv.axon_hooks, which this image lacks; register the equivalent hook
    from trn_agent_boot so tracing works instead of crashing."""
    try:
        import antenv.axon_hooks  # noqa: F401
        return
    except ImportError:
        pass
    try:
        from trn_agent_boot.trn_boot import _ntff_profile_via_ctypes

        hook = _ntff_profile_via_ctypes("/opt/axon/libaxon_pjrt.so")
    except Exception:
        hook = None
    mod = types.ModuleType("antenv.axon_hooks")
    mod.get_axon_ntff_profile_hook = lambda: hook
    mod.set_axon_ntff_profile_hook = lambda h: None
    sys.modules["antenv.axon_hooks"] = mod


_install_ntff_hook()

F32 = mybir.dt.float32
F32R = mybir.dt.float32r
AF = mybir.ActivationFunctionType
OP = mybir.AluOpType

P = 128
CI = 256  # input channels (2 chunks of 128)
CINT = 128  # intermediate channels
CO = 256  # output channels (2 blocks of 128)
N = 4096  # key/value positions (32 blocks of 128)
Q = 2048  # queries per core
B, T, H, W = 4, 4, 32, 32
NKB = N // P  # 32 key blocks

# dtype used for matmul operands (fp32 data produced as float32r runs the PE
# at full rate for free dims >= 256; plain float32 runs at 1/4 rate)
MM_DT = F32R


def build():
    nc = bacc.Bacc(None, target_bir_lowering=False, debug=False)

    xb = nc.dram_tensor("xb", [CI, N], F32, kind="ExternalInput").ap()
    # all weights/constants packed host-side into one array -> one DMA; the
    # projection weights arrive PRE-TRANSPOSED (host numpy):
    # cols [0:256]=wtT, [256:512]=wpT, [512:768]=wgT, [768:1024]=woT,
    # [1024:1152]=identity(f32r), [1152:1280]=ones, [1280:1285]=biases
    cpak = nc.dram_tensor("cpak", [P, 1285], F32, kind="ExternalInput").ap()
    oq = nc.dram_tensor("oq", [CO, Q], F32, kind="ExternalOutput").ap()

    with tile.TileContext(nc) as tc:
        with (
            tc.tile_pool(name="consts", bufs=1) as consts,
            tc.tile_pool(name="big", bufs=1) as big,
            tc.tile_pool(name="tmp", bufs=6) as tmp,
        ):
            # ---- constants on the sync queue; x chunks spread across the
            # scalar/vector/gpsimd queues in need-order so the first
            # projection can start as early as possible ----
            cpak_sb = consts.tile([P, 1285], MM_DT, tag="cpak")
            nc.sync.dma_start(cpak_sb[:], cpak.bitcast(MM_DT))
            wtT = cpak_sb[:, 0:256].rearrange("p (o c) -> p o c", o=2)
            wpT = cpak_sb[:, 256:512].rearrange("p (o c) -> p o c", o=2)
            wgT = cpak_sb[:, 512:768].rearrange("p (o c) -> p o c", o=2)
            woT = cpak_sb[:, 768:1024].rearrange("p (o c) -> p o c", o=2)
            identity_r = cpak_sb[:, 1024:1152]
            ones_sb = cpak_sb[:, 1152:1280]
            bt_sb = cpak_sb[:, 1280:1281].bitcast(F32)
            bp_sb = cpak_sb[:, 1281:1282].bitcast(F32)
            bg_sb = cpak_sb[:, 1282:1283].bitcast(F32)
            bo_sb = cpak_sb[:, 1283:1285].bitcast(F32)

            x_sb = big.tile([P, 2, N], MM_DT, tag="x")
            xbr = xb.rearrange("(o p) n -> p o n", p=P).bitcast(MM_DT)
            # need-ordered x chunks: HWDGE queues (scalar/sync) for the
            # early columns, the slow-starting SWDGE (gpsimd) queue only for
            # the late-needed tail
            for eng, sl in (
                (nc.scalar, slice(0, 512)),
                (nc.sync, slice(512, 1024)),
                (nc.scalar, slice(1024, 2048)),
                (nc.gpsimd, slice(2048, 3072)),
                (nc.gpsimd, slice(3072, 4096)),
            ):
                eng.dma_start(x_sb[:, :, sl], xbr[:, :, sl])

            # SBUF buffers shared across phases
            theta_sb = big.tile([P, Q], MM_DT, tag="theta")
            phi_sb = big.tile([P, N], MM_DT, tag="phi")
            g_sb = big.tile([P, N], MM_DT, tag="g")
            gT_sb = big.tile([P, N], MM_DT, tag="gT")  # kb-blocked g^T
            y_sb = big.tile([P, Q], MM_DT, tag="y")
            d_acc = big.tile([P, Q], MM_DT, tag="dacc")
            out_sb = big.tile([P, 2, Q], F32, tag="out")
            oqr = oq.rearrange("(o p) q -> p o q", p=P)

            def make_attn_group(ps_s, ps_y, ps_proj=None, work=None):
                def attn_group(gi, q0, qw, pending_out=None, final=False):
                    """One query group's attention.  scores/y on PE, exp on
                    Act, softmax denominator accumulated on DVE (d_acc +=
                    exp tile).  Deferred projection pieces (`work`, group 0)
                    and the previous group's output projection (pending_out,
                    one piece per kb) are interleaved to fill engine gaps.
                    Returns a list of output-projection piece closures."""
                    qsl = slice(q0, q0 + qw)
                    nh = qw // 512
                    with nc.named_scope(f"attn{gi}"):
                        y_ps = ps_y.tile([P, qw], F32, tag="y", name=f"y_ps{gi}")

                        def scores(kb):
                            s_ps = ps_s.tile(
                                [P, qw], F32, tag="s", name=f"s{gi}_{kb}"
                            )
                            for h in range(nh):
                                nc.tensor.matmul(
                                    s_ps[:, h * 512 : (h + 1) * 512],
                                    phi_sb[:, kb * P : (kb + 1) * P],
                                    theta_sb[:, q0 + h * 512 : q0 + (h + 1) * 512],
                                    start=True, stop=True,
                                )
                            return s_ps

                        s_cur = scores(0)
                        for kb in range(NKB):
                            at = tmp.tile(
                                [P, qw], MM_DT, tag="attn", name=f"at{gi}_{kb}"
                            )
                            if final and kb == NKB - 1 and nh > 1:
                                # last exp of the kernel: split per 512 so
                                # the tail epilogue starts sooner
                                for h in range(nh):
                                    hsl = slice(h * 512, (h + 1) * 512)
                                    nc.scalar.activation(
                                        out=at[:, hsl], in_=s_cur[:, hsl],
                                        func=AF.Exp,
                                    )
                            else:
                                nc.scalar.activation(
                                    out=at[:], in_=s_cur[:], func=AF.Exp
                                )
                            if kb + 1 < NKB:
                                # feed the PE the next scores before y(kb) so
                                # it is not idle while Act runs exp(kb)
                                s_cur = scores(kb + 1)
                            # one deferred projection piece per kb (group 0)
                            if work:
                                fn, arg = work.pop(0)
                                fn(*arg)
                            first, last = kb == 0, kb == NKB - 1
                            for h in range(nh):
                                hsl = slice(h * 512, (h + 1) * 512)
                                nc.tensor.matmul(
                                    y_ps[:, hsl], gT_sb[:, kb * P : (kb + 1) * P],
                                    at[:, hsl], start=first, stop=last,
                                )
                            # softmax denominator on DVE
                            if kb == 0:
                                nc.vector.tensor_copy(
                                    out=d_acc[:, qsl], in_=at[:]
                                )
                            else:
                                nc.vector.tensor_tensor(
                                    out=d_acc[:, qsl], in0=d_acc[:, qsl],
                                    in1=at[:], op=OP.add,
                                )
                            # previous group's output projection, one piece
                            # per kb starting at kb=2 (spreads the PSUM-slot
                            # churn instead of a serializing burst)
                            if pending_out and kb >= 2:
                                pending_out.pop(0)(ps_s)
                        while pending_out:
                            pending_out.pop(0)(ps_s)
                        # evacuate y before this group's PSUM scope can be
                        # torn down (the deferred pieces run in the next
                        # group's scope); Act is between exp streams here
                        for h in range(nh):
                            hsl = slice(h * 512, (h + 1) * 512)
                            qhsl = slice(q0 + h * 512, q0 + (h + 1) * 512)
                            nc.scalar.activation(
                                out=y_sb[:, qhsl], in_=y_ps[:, hsl], func=AF.Copy
                            )

                    pieces = []
                    rd = tmp.tile([P, qw], F32, tag="rd", name=f"rd{gi}")

                    def epi_h(pool, h):
                        # per-512 epilogue: denominator broadcast
                        # (ones-matmul) and reciprocal
                        hsl = slice(h * 512, (h + 1) * 512)
                        qhsl = slice(q0 + h * 512, q0 + (h + 1) * 512)
                        with nc.named_scope(f"epi{gi}"):
                            d_bc = pool.tile(
                                [P, 512], F32, tag="s", name=f"dbc{gi}{h}"
                            )
                            nc.tensor.matmul(
                                d_bc[:], ones_sb, d_acc[:, qhsl],
                                start=True, stop=True,
                            )
                            nc.vector.reciprocal_approx_fast(
                                out=rd[:, hsl], in_=d_bc[:],
                            )

                    def out_piece(pool, blk, h):
                        hsl = slice(h * 512, (h + 1) * 512)
                        qhsl = slice(q0 + h * 512, q0 + (h + 1) * 512)
                        with nc.named_scope(f"outp{gi}"):
                            po = pool.tile(
                                [P, 512], F32, tag="s", name=f"po{gi}{blk}{h}"
                            )
                            nc.tensor.matmul(
                                po[:], woT[:, blk, :], y_sb[:, qhsl],
                                start=True, stop=True,
                            )
                            # out = (po + b_out_eff) * rd in one DVE pass
                            nc.vector.scalar_tensor_tensor(
                                out=out_sb[:, blk, qhsl], in0=po[:],
                                scalar=bo_sb[:, blk : blk + 1], in1=rd[:, hsl],
                                op0=OP.add, op1=OP.mult,
                            )
                            nc.sync.dma_start(
                                oqr[:, blk, qhsl], out_sb[:, blk, qhsl]
                            )

                    # h-major so each half's chain drains independently
                    for h in range(nh):
                        pieces.append(lambda pool, h=h: epi_h(pool, h))
                        for blk in range(2):
                            pieces.append(
                                lambda pool, blk=blk, h=h: out_piece(pool, blk, h)
                            )
                    if final:
                        for p in pieces:
                            p(ps_s)
                        return []
                    return pieces

                return attn_group

            with (
                tc.tile_pool(name="ps_sA", bufs=2, space="PSUM") as ps_sA,
                tc.tile_pool(name="ps_yA", bufs=1, space="PSUM") as ps_yA,
                tc.tile_pool(name="ps_proj", bufs=2, space="PSUM") as ps_proj,
            ):
                # ---- projection emitters ----------------------------------
                def proj(which, j, on_act=False):
                    wT, bias, dst = {
                        "t": (wtT, bt_sb, theta_sb),
                        "p": (wpT, bp_sb, phi_sb),
                        "g": (wgT, bg_sb, g_sb),
                    }[which]
                    sl = slice(j * 512, (j + 1) * 512)
                    pp = ps_proj.tile([P, 512], F32, tag="pp", name=f"pp{which}{j}")
                    nc.tensor.matmul(
                        pp[:], wT[:, 0, :], x_sb[:, 0, sl], start=True, stop=False
                    )
                    nc.tensor.matmul(
                        pp[:], wT[:, 1, :], x_sb[:, 1, sl], start=False, stop=True
                    )
                    if on_act:
                        # only used in the prologue, before the exp stream
                        nc.scalar.activation(
                            out=dst[:, sl], in_=pp[:], func=AF.Identity,
                            bias=bias,
                        )
                    else:
                        nc.vector.tensor_scalar(
                            out=dst[:, sl], in0=pp[:],
                            scalar1=bias, scalar2=None, op0=OP.add,
                        )

                def gtq(j, on_act=False):
                    # transpose g columns 512j..512j+512 (4 key blocks) into
                    # bitcast slices of one projection-ring PSUM tile, then
                    # one copy into gT.  (f32r transposes run at 1.5
                    # cycles/row; direct gT-from-x matmuls would be 128-wide
                    # f32r = quarter rate, slower.)
                    sl = slice(j * 512, (j + 1) * 512)
                    pq = ps_proj.tile([P, 512], F32, tag="pp", name=f"pq{j}")
                    for k in range(4):
                        ksl = slice(j * 512 + k * P, j * 512 + (k + 1) * P)
                        nc.tensor.transpose(
                            pq[:, k * P : (k + 1) * P].bitcast(MM_DT),
                            g_sb[:, ksl], identity_r,
                        )
                    if on_act:
                        nc.scalar.activation(
                            out=gT_sb[:, sl], in_=pq[:].bitcast(MM_DT),
                            func=AF.Copy,
                        )
                    else:
                        nc.vector.tensor_copy(
                            out=gT_sb[:, sl], in_=pq[:].bitcast(MM_DT)
                        )

                # deferred pieces, one per kb of group 0.  phi j feeds
                # scores kb=4j (emitted one kb early); gT quad j feeds y
                # kb=4j; theta j2/j3 feed group 1.
                work = [
                    (proj, ("p", 1)), (proj, ("g", 1)), (proj, ("t", 2)),
                    (gtq, (1,)),
                ]
                for j in range(2, 8):
                    work += [(proj, ("p", j)), (proj, ("g", j)), (gtq, (j,))]
                work += [(proj, ("t", 3))]
                # deadline check: piece k emits at kb=k.  quad j at 3j <=
                # its deadline y(4j) (quad1 at 3 <= 4); phi j at 3j-2 <=
                # 4j-1.  ok.

                # prologue: minimum for scores/y at kb=0..3, evacuated on
                # the still-idle Act engine
                proj("t", 0, on_act=True)
                proj("p", 0, on_act=True)
                proj("g", 0, on_act=True)
                gtq(0, on_act=True)
                proj("t", 1, on_act=True)

                grpA = make_attn_group(ps_sA, ps_yA, ps_proj, work)
                out0 = grpA(0, 0, 1024)

            with (
                tc.tile_pool(name="ps_sB", bufs=3, space="PSUM") as ps_sB,
                tc.tile_pool(name="ps_yB", bufs=1, space="PSUM") as ps_yB,
            ):
                grpB = make_attn_group(ps_sB, ps_yB)
                grpB(1, 1024, 1024, pending_out=out0, final=True)

    nc.compile()
    return nc


IDENT = np.eye(P, dtype=np.float32)

_NC_CACHE = None
LAST_EXEC_TIME_NS = None
LAST_TRACE = None
LAST_RESULTS = None


def _get_nc():
    global _NC_CACHE
    if _NC_CACHE is None:
        _NC_CACHE = build()
    return _NC_CACHE


def kernel(**inputs):
    x = np.ascontiguousarray(np.asarray(inputs["x"], dtype=np.float32))
    assert x.shape == (B, CI, T, H, W), x.shape
    xf = x.reshape(B, CI, N)
    w = {
        k: np.ascontiguousarray(np.asarray(inputs[k], dtype=np.float32))
        for k in (
            "w_theta", "b_theta", "w_phi", "b_phi", "w_g", "b_g", "w_out", "b_out"
        )
    }

    def proj_t(wm):
        # [p, o*128+c] = wm[c, o*128+p]
        return wm.T.reshape(2, P, P).transpose(1, 0, 2).reshape(P, 2 * P)

    woT_h = w["w_out"].reshape(2, P, CINT).transpose(2, 0, 1).reshape(P, 2 * P)
    CPAK = np.ascontiguousarray(
        np.concatenate(
            [
                proj_t(w["w_theta"]), proj_t(w["w_phi"]), proj_t(w["w_g"]),
                woT_h, IDENT, np.ones((P, P), np.float32),
                np.stack(
                    [
                        w["b_theta"], w["b_phi"], w["b_g"],
                        w["b_out"][:P], w["b_out"][P:],
                    ],
                    axis=1,
                ),
            ],
            axis=1,
        )
    )
    in_maps = []
    for core in range(8):
        b, h = core // 2, core % 2
        if h == 0:
            xcore = xf[b]
        else:
            xcore = np.ascontiguousarray(
                np.concatenate([xf[b][:, Q:], xf[b][:, :Q]], axis=1)
            )
        in_maps.append(
            {"xb": xcore, "cpak": CPAK}
        )

    nc = _get_nc()
    res = run_bass_kernel_spmd(nc, in_maps, core_ids=list(range(8)))
    global LAST_EXEC_TIME_NS, LAST_TRACE, LAST_RESULTS
    LAST_EXEC_TIME_NS = res.exec_time_ns
    LAST_TRACE = res.instructions_and_trace[1] if res.instructions_and_trace else None
    LAST_RESULTS = res

    out = np.empty((B, CO, N), np.float32)
    for core in range(8):
        b, h = core // 2, core % 2
        out[b][:, h * Q : (h + 1) * Q] = res.results[core]["oq"]
    return out.reshape(B, CO, T, H, W)


# revision 20
# speedup vs baseline: 1.1560x; 1.0051x over previous
"""Non-local block (B=4, C_in=256, C_int=128, C_out=256, N=T*H*W=4096) on 8
Trainium2 NeuronCores.

Sharding: data-parallel over batch (4 batches) x query-halves (2) = 8 cores.
Each core holds one batch's full x (for keys/values); the host rotates each
core's columns so its 2048 queries are always columns 0:2048 (attention is
permutation-invariant over keys). Per core: theta/phi/g projections, the
[2048q x 4096k] attention with softmax (keys on partitions), and the output
projection for its query half. Host gathers the 8 [256, 2048] slices.

Engine layout: PE does scores+y+projections; Act does the 64 [128,1024] exp
tiles (~1.0us each -- the pace-setter); DVE accumulates the softmax
denominator as elementwise adds over the exp tiles (d_acc += at), replacing
the ones-matmuls that burned ~27us of PE in the old version; one tiny
stationary-ones matmul per group broadcasts 1/d at output time.  Pool
(gpsimd) evacuates projection/gT/y PSUM.  Projections are interleaved INTO
the attention stream so exp starts as soon as the first x chunk lands
instead of after all projections.

PSUM budget (8 banks): scores ring 2x[128,1024]=4, y accumulator
1x[128,1024]=2, projection ring 2x[128,512]=2 (transposes write bitcast
slices of the projection tiles).
"""

import sys
import types

import numpy as np

import concourse.bacc as bacc
import concourse.mybir as mybir
import concourse.tile as tile
from concourse.bass_utils import run_bass_kernel_spmd


def _install_ntff_hook():
    """If tracing is requested (BASS_TRACE=1) under axon, bass_utils imports
    antenv.axon_hooks, which this image lacks; register the equivalent hook
    from trn_agent_boot so tracing works instead of crashing."""
    try:
        import antenv.axon_hooks  # noqa: F401
        return
    except ImportError:
        pass
    try:
        from trn_agent_boot.trn_boot import _ntff_profile_via_ctypes

        hook = _ntff_profile_via_ctypes("/opt/axon/libaxon_pjrt.so")
    except Exception:
        hook = None
    mod = types.ModuleType("antenv.axon_hooks")
    mod.get_axon_ntff_profile_hook = lambda: hook
    mod.set_axon_ntff_profile_hook = lambda h: None
    sys.modules["antenv.axon_hooks"] = mod


_install_ntff_hook()

F32 = mybir.dt.float32
F32R = mybir.dt.float32r
AF = mybir.ActivationFunctionType
OP = mybir.AluOpType

P = 128
CI = 256  # input channels (2 chunks of 128)
CINT = 128  # intermediate channels
CO = 256  # output channels (2 blocks of 128)
N = 4096  # key/value positions (32 blocks of 128)
Q = 2048  # queries per core
B, T, H, W = 4, 4, 32, 32
NKB = N // P  # 32 key blocks

# dtype used for matmul operands (fp32 data produced as float32r runs the PE
# at full rate for free dims >= 256; plain float32 runs at 1/4 rate)
MM_DT = F32R


def build():
    nc = bacc.Bacc(None, target_bir_lowering=False, debug=False)

    xb = nc.dram_tensor("xb", [CI, N], F32, kind="ExternalInput").ap()
    # all weights/constants packed host-side into one array -> one DMA; the
    # projection weights arrive PRE-TRANSPOSED (host numpy):
    # cols [0:256]=wtT, [256:512]=wpT, [512:768]=wgT, [768:1024]=woT,
    # [1024:1152]=identity(f32r), [1152:1280]=ones, [1280:1285]=biases
    cpak = nc.dram_tensor("cpak", [P, 1285], F32, kind="ExternalInput").ap()
    oq = nc.dram_tensor("oq", [CO, Q], F32, kind="ExternalOutput").ap()

    with tile.TileContext(nc) as tc:
        with (
            tc.tile_pool(name="consts", bufs=1) as consts,
            tc.tile_pool(name="big", bufs=1) as big,
            tc.tile_pool(name="tmp", bufs=6) as tmp,
        ):
            # ---- constants on the sync queue; x chunks spread across the
            # scalar/vector/gpsimd queues in need-order so the first
            # projection can start as early as possible ----
            cpak_sb = consts.tile([P, 1285], MM_DT, tag="cpak")
            nc.sync.dma_start(cpak_sb[:], cpak.bitcast(MM_DT))
            wtT = cpak_sb[:, 0:256].rearrange("p (o c) -> p o c", o=2)
            wpT = cpak_sb[:, 256:512].rearrange("p (o c) -> p o c", o=2)
            wgT = cpak_sb[:, 512:768].rearrange("p (o c) -> p o c", o=2)
            woT = cpak_sb[:, 768:1024].rearrange("p (o c) -> p o c", o=2)
            identity_r = cpak_sb[:, 1024:1152]
            ones_sb = cpak_sb[:, 1152:1280]
            bt_sb = cpak_sb[:, 1280:1281].bitcast(F32)
            bp_sb = cpak_sb[:, 1281:1282].bitcast(F32)
            bg_sb = cpak_sb[:, 1282:1283].bitcast(F32)
            bo_sb = cpak_sb[:, 1283:1285].bitcast(F32)

            x_sb = big.tile([P, 2, N], MM_DT, tag="x")
            xbr = xb.rearrange("(o p) n -> p o n", p=P).bitcast(MM_DT)
            # need-ordered x chunks; the startup critical path is cpak+x0:
            # serialize the next-needed chunks behind them on the sync queue
            # so they don't steal DMA bandwidth, and gate the late tail
            # chunks (gpsimd/SWDGE) behind the first theta evacuation via a
            # tiny copy so their transfers start only after the critical
            # window
            nc.scalar.dma_start(x_sb[:, :, 0:512], xbr[:, :, 0:512])
            nc.sync.dma_start(x_sb[:, :, 512:1024], xbr[:, :, 512:1024])
            nc.sync.dma_start(x_sb[:, :, 1024:2048], xbr[:, :, 1024:2048])

            # SBUF buffers shared across phases
            theta_sb = big.tile([P, Q], MM_DT, tag="theta")
            phi_sb = big.tile([P, N], MM_DT, tag="phi")
            g_sb = big.tile([P, N], MM_DT, tag="g")
            gT_sb = big.tile([P, N], MM_DT, tag="gT")  # kb-blocked g^T
            y_sb = big.tile([P, Q], MM_DT, tag="y")
            d_acc = big.tile([P, Q], MM_DT, tag="dacc")
            out_sb = big.tile([P, 2, Q], F32, tag="out")
            oqr = oq.rearrange("(o p) q -> p o q", p=P)

            def make_attn_group(ps_s, ps_y, ps_proj=None, work=None):
                def attn_group(gi, q0, qw, pending_out=None, final=False):
                    """One query group's attention.  scores/y on PE, exp on
                    Act, softmax denominator accumulated on DVE (d_acc +=
                    exp tile).  Deferred projection pieces (`work`, group 0)
                    and the previous group's output projection (pending_out,
                    one piece per kb) are interleaved to fill engine gaps.
                    Returns a list of output-projection piece closures."""
                    qsl = slice(q0, q0 + qw)
                    nh = qw // 512
                    with nc.named_scope(f"attn{gi}"):
                        y_ps = ps_y.tile([P, qw], F32, tag="y", name=f"y_ps{gi}")

                        def scores(kb):
                            s_ps = ps_s.tile(
                                [P, qw], F32, tag="s", name=f"s{gi}_{kb}"
                            )
                            for h in range(nh):
                                nc.tensor.matmul(
                                    s_ps[:, h * 512 : (h + 1) * 512],
                                    phi_sb[:, kb * P : (kb + 1) * P],
                                    theta_sb[:, q0 + h * 512 : q0 + (h + 1) * 512],
                                    start=True, stop=True,
                                )
                            return s_ps

                        s_cur = scores(0)
                        for kb in range(NKB):
                            at = tmp.tile(
                                [P, qw], MM_DT, tag="attn", name=f"at{gi}_{kb}"
                            )
                            if final and kb == NKB - 1 and nh > 1:
                                # last exp of the kernel: split per 512 so
                                # the tail epilogue starts sooner
                                for h in range(nh):
                                    hsl = slice(h * 512, (h + 1) * 512)
                                    nc.scalar.activation(
                                        out=at[:, hsl], in_=s_cur[:, hsl],
                                        func=AF.Exp,
                                    )
                            else:
                                nc.scalar.activation(
                                    out=at[:], in_=s_cur[:], func=AF.Exp
                                )
                            if kb + 1 < NKB:
                                # feed the PE the next scores before y(kb) so
                                # it is not idle while Act runs exp(kb)
                                s_cur = scores(kb + 1)
                            # one deferred projection piece per kb (group 0)
                            if work:
                                fn, arg = work.pop(0)
                                fn(*arg)
                            first, last = kb == 0, kb == NKB - 1
                            for h in range(nh):
                                hsl = slice(h * 512, (h + 1) * 512)
                                nc.tensor.matmul(
                                    y_ps[:, hsl], gT_sb[:, kb * P : (kb + 1) * P],
                                    at[:, hsl], start=first, stop=last,
                                )
                            # softmax denominator on DVE
                            if kb == 0:
                                nc.vector.tensor_copy(
                                    out=d_acc[:, qsl], in_=at[:]
                                )
                            else:
                                nc.vector.tensor_tensor(
                                    out=d_acc[:, qsl], in0=d_acc[:, qsl],
                                    in1=at[:], op=OP.add,
                                )
                            # previous group's output projection, one piece
                            # per kb starting at kb=2 (spreads the PSUM-slot
                            # churn instead of a serializing burst)
                            if pending_out and kb >= 2:
                                pending_out.pop(0)(ps_s)
                        while pending_out:
                            pending_out.pop(0)(ps_s)
                        # evacuate y before this group's PSUM scope can be
                        # torn down (the deferred pieces run in the next
                        # group's scope); Act is between exp streams here
                        for h in range(nh):
                            hsl = slice(h * 512, (h + 1) * 512)
                            qhsl = slice(q0 + h * 512, q0 + (h + 1) * 512)
                            nc.scalar.activation(
                                out=y_sb[:, qhsl], in_=y_ps[:, hsl], func=AF.Copy
                            )

                    pieces = []
                    rd = tmp.tile([P, qw], F32, tag="rd", name=f"rd{gi}")

                    def epi_h(pool, h):
                        # per-512 epilogue: denominator broadcast
                        # (ones-matmul) and reciprocal
                        hsl = slice(h * 512, (h + 1) * 512)
                        qhsl = slice(q0 + h * 512, q0 + (h + 1) * 512)
                        with nc.named_scope(f"epi{gi}"):
                            d_bc = pool.tile(
                                [P, 512], F32, tag="s", name=f"dbc{gi}{h}"
                            )
                            nc.tensor.matmul(
                                d_bc[:], ones_sb, d_acc[:, qhsl],
                                start=True, stop=True,
                            )
                            nc.vector.reciprocal_approx_fast(
                                out=rd[:, hsl], in_=d_bc[:],
                            )

                    def out_piece(pool, blk, h):
                        hsl = slice(h * 512, (h + 1) * 512)
                        qhsl = slice(q0 + h * 512, q0 + (h + 1) * 512)
                        with nc.named_scope(f"outp{gi}"):
                            po = pool.tile(
                                [P, 512], F32, tag="s", name=f"po{gi}{blk}{h}"
                            )
                            nc.tensor.matmul(
                                po[:], woT[:, blk, :], y_sb[:, qhsl],
                                start=True, stop=True,
                            )
                            # out = (po + b_out_eff) * rd in one DVE pass
                            nc.vector.scalar_tensor_tensor(
                                out=out_sb[:, blk, qhsl], in0=po[:],
                                scalar=bo_sb[:, blk : blk + 1], in1=rd[:, hsl],
                                op0=OP.add, op1=OP.mult,
                            )
                            nc.sync.dma_start(
                                oqr[:, blk, qhsl], out_sb[:, blk, qhsl]
                            )

                    # h-major so each half's chain drains independently
                    for h in range(nh):
                        pieces.append(lambda pool, h=h: epi_h(pool, h))
                        for blk in range(2):
                            pieces.append(
                                lambda pool, blk=blk, h=h: out_piece(pool, blk, h)
                            )
                    if final:
                        for p in pieces:
                            p(ps_s)
                        return []
                    return pieces

                return attn_group

            with (
                tc.tile_pool(name="ps_sA", bufs=2, space="PSUM") as ps_sA,
                tc.tile_pool(name="ps_yA", bufs=1, space="PSUM") as ps_yA,
                tc.tile_pool(name="ps_proj", bufs=2, space="PSUM") as ps_proj,
            ):
                # ---- projection emitters ----------------------------------
                def proj(which, j, on_act=False):
                    wT, bias, dst = {
                        "t": (wtT, bt_sb, theta_sb),
                        "p": (wpT, bp_sb, phi_sb),
                        "g": (wgT, bg_sb, g_sb),
                    }[which]
                    sl = slice(j * 512, (j + 1) * 512)
                    pp = ps_proj.tile([P, 512], F32, tag="pp", name=f"pp{which}{j}")
                    nc.tensor.matmul(
                        pp[:], wT[:, 0, :], x_sb[:, 0, sl], start=True, stop=False
                    )
                    nc.tensor.matmul(
                        pp[:], wT[:, 1, :], x_sb[:, 1, sl], start=False, stop=True
                    )
                    if on_act:
                        # only used in the prologue, before the exp stream
                        nc.scalar.activation(
                            out=dst[:, sl], in_=pp[:], func=AF.Identity,
                            bias=bias,
                        )
                    else:
                        nc.vector.tensor_scalar(
                            out=dst[:, sl], in0=pp[:],
                            scalar1=bias, scalar2=None, op0=OP.add,
                        )

                def gtq(j, on_act=False):
                    # transpose g columns 512j..512j+512 (4 key blocks) into
                    # bitcast slices of one projection-ring PSUM tile, then
                    # one copy into gT.  (f32r transposes run at 1.5
                    # cycles/row; direct gT-from-x matmuls would be 128-wide
                    # f32r = quarter rate, slower.)
                    sl = slice(j * 512, (j + 1) * 512)
                    pq = ps_proj.tile([P, 512], F32, tag="pp", name=f"pq{j}")
                    for k in range(4):
                        ksl = slice(j * 512 + k * P, j * 512 + (k + 1) * P)
                        nc.tensor.transpose(
                            pq[:, k * P : (k + 1) * P].bitcast(MM_DT),
                            g_sb[:, ksl], identity_r,
                        )
                    if on_act:
                        nc.scalar.activation(
                            out=gT_sb[:, sl], in_=pq[:].bitcast(MM_DT),
                            func=AF.Copy,
                        )
                    else:
                        nc.vector.tensor_copy(
                            out=gT_sb[:, sl], in_=pq[:].bitcast(MM_DT)
                        )

                # deferred pieces, one per kb of group 0.  phi j feeds
                # scores kb=4j (emitted one kb early); gT quad j feeds y
                # kb=4j; theta j2/j3 feed group 1.
                work = [
                    (proj, ("p", 1)), (proj, ("g", 1)), (proj, ("t", 2)),
                    (gtq, (1,)),
                ]
                for j in range(2, 8):
                    work += [(proj, ("p", j)), (proj, ("g", j)), (gtq, (j,))]
                work += [(proj, ("t", 3))]
                # deadline check: piece k emits at kb=k.  quad j at 3j <=
                # its deadline y(4j) (quad1 at 3 <= 4); phi j at 3j-2 <=
                # 4j-1.  ok.

                # prologue: minimum for scores/y at kb=0..3, evacuated on
                # the still-idle Act engine
                proj("t", 0, on_act=True)
                dma_gate = big.tile([P, 1], MM_DT, tag="gate")
                nc.gpsimd.tensor_copy(out=dma_gate[:], in_=theta_sb[:, 0:1])
                nc.gpsimd.dma_start(
                    x_sb[:, :, 2048:3072], xbr[:, :, 2048:3072]
                )
                nc.gpsimd.dma_start(
                    x_sb[:, :, 3072:4096], xbr[:, :, 3072:4096]
                )
                proj("p", 0, on_act=True)
                proj("g", 0, on_act=True)
                gtq(0, on_act=True)
                proj("t", 1, on_act=True)

                grpA = make_attn_group(ps_sA, ps_yA, ps_proj, work)
                out0 = grpA(0, 0, 1024)

            with (
                tc.tile_pool(name="ps_sB", bufs=3, space="PSUM") as ps_sB,
                tc.tile_pool(name="ps_yB", bufs=1, space="PSUM") as ps_yB,
            ):
                grpB = make_attn_group(ps_sB, ps_yB)
                grpB(1, 1024, 1024, pending_out=out0, final=True)

    nc.compile()
    return nc


IDENT = np.eye(P, dtype=np.float32)

_NC_CACHE = None
LAST_EXEC_TIME_NS = None
LAST_TRACE = None
LAST_RESULTS = None


def _get_nc():
    global _NC_CACHE
    if _NC_CACHE is None:
        _NC_CACHE = build()
    return _NC_CACHE


def kernel(**inputs):
    x = np.ascontiguousarray(np.asarray(inputs["x"], dtype=np.float32))
    assert x.shape == (B, CI, T, H, W), x.shape
    xf = x.reshape(B, CI, N)
    w = {
        k: np.ascontiguousarray(np.asarray(inputs[k], dtype=np.float32))
        for k in (
            "w_theta", "b_theta", "w_phi", "b_phi", "w_g", "b_g", "w_out", "b_out"
        )
    }

    def proj_t(wm):
        # [p, o*128+c] = wm[c, o*128+p]
        return wm.T.reshape(2, P, P).transpose(1, 0, 2).reshape(P, 2 * P)

    woT_h = w["w_out"].reshape(2, P, CINT).transpose(2, 0, 1).reshape(P, 2 * P)
    CPAK = np.ascontiguousarray(
        np.concatenate(
            [
                proj_t(w["w_theta"]), proj_t(w["w_phi"]), proj_t(w["w_g"]),
                woT_h, IDENT, np.ones((P, P), np.float32),
                np.stack(
                    [
                        w["b_theta"], w["b_phi"], w["b_g"],
                        w["b_out"][:P], w["b_out"][P:],
                    ],
                    axis=1,
                ),
            ],
            axis=1,
        )
    )
    in_maps = []
    for core in range(8):
        b, h = core // 2, core % 2
        if h == 0:
            xcore = xf[b]
        else:
            xcore = np.ascontiguousarray(
                np.concatenate([xf[b][:, Q:], xf[b][:, :Q]], axis=1)
            )
        in_maps.append(
            {"xb": xcore, "cpak": CPAK}
        )

    nc = _get_nc()
    res = run_bass_kernel_spmd(nc, in_maps, core_ids=list(range(8)))
    global LAST_EXEC_TIME_NS, LAST_TRACE, LAST_RESULTS
    LAST_EXEC_TIME_NS = res.exec_time_ns
    LAST_TRACE = res.instructions_and_trace[1] if res.instructions_and_trace else None
    LAST_RESULTS = res

    out = np.empty((B, CO, N), np.float32)
    for core in range(8):
        b, h = core // 2, core % 2
        out[b][:, h * Q : (h + 1) * Q] = res.results[core]["oq"]
    return out.reshape(B, CO, T, H, W)


# revision 21
# speedup vs baseline: 1.2048x; 1.0422x over previous
"""Non-local block (B=4, C_in=256, C_int=128, C_out=256, N=T*H*W=4096) on 8
Trainium2 NeuronCores.

Sharding: data-parallel over batch (4 batches) x query-halves (2) = 8 cores.
Each core holds one batch's full x (for keys/values); the host rotates each
core's columns so its 2048 queries are always columns 0:2048 (attention is
permutation-invariant over keys). Per core: theta/phi/g projections, the
[2048q x 4096k] attention with softmax (keys on partitions), and the output
projection for its query half. Host gathers the 8 [256, 2048] slices.

Engine layout: PE does scores+y+projections; Act does the 64 [128,1024] exp
tiles (~1.0us each -- the pace-setter); DVE accumulates the softmax
denominator as elementwise adds over the exp tiles (d_acc += at), replacing
the ones-matmuls that burned ~27us of PE in the old version; one tiny
stationary-ones matmul per group broadcasts 1/d at output time.  Pool
(gpsimd) evacuates projection/gT/y PSUM.  Projections are interleaved INTO
the attention stream so exp starts as soon as the first x chunk lands
instead of after all projections.

PSUM budget (8 banks): scores ring 2x[128,1024]=4, y accumulator
1x[128,1024]=2, projection ring 2x[128,512]=2 (transposes write bitcast
slices of the projection tiles).
"""

import sys
import types

import numpy as np

import concourse.bacc as bacc
import concourse.mybir as mybir
import concourse.tile as tile
from concourse.bass_utils import run_bass_kernel_spmd


def _install_ntff_hook():
    """If tracing is requested (BASS_TRACE=1) under axon, bass_utils imports
    antenv.axon_hooks, which this image lacks; register the equivalent hook
    from trn_agent_boot so tracing works instead of crashing."""
    try:
        import antenv.axon_hooks  # noqa: F401
        return
    except ImportError:
        pass
    try:
        from trn_agent_boot.trn_boot import _ntff_profile_via_ctypes

        hook = _ntff_profile_via_ctypes("/opt/axon/libaxon_pjrt.so")
    except Exception:
        hook = None
    mod = types.ModuleType("antenv.axon_hooks")
    mod.get_axon_ntff_profile_hook = lambda: hook
    mod.set_axon_ntff_profile_hook = lambda h: None
    sys.modules["antenv.axon_hooks"] = mod


_install_ntff_hook()

F32 = mybir.dt.float32
F32R = mybir.dt.float32r
AF = mybir.ActivationFunctionType
OP = mybir.AluOpType

P = 128
CI = 256  # input channels (2 chunks of 128)
CINT = 128  # intermediate channels
CO = 256  # output channels (2 blocks of 128)
N = 4096  # key/value positions (32 blocks of 128)
Q = 2048  # queries per core
B, T, H, W = 4, 4, 32, 32
NKB = N // P  # 32 key blocks

# dtype used for matmul operands (fp32 data produced as float32r runs the PE
# at full rate for free dims >= 256; plain float32 runs at 1/4 rate)
MM_DT = F32R


def build():
    nc = bacc.Bacc(None, target_bir_lowering=False, debug=False)

    xb = nc.dram_tensor("xb", [CI, N], F32, kind="ExternalInput").ap()
    # all weights/constants packed host-side into one array -> one DMA; the
    # projection weights arrive PRE-TRANSPOSED (host numpy):
    # cols [0:256]=wtT, [256:512]=wpT, [512:768]=wgT, [768:1024]=woT,
    # [1024:1152]=identity(f32r), [1152:1280]=ones, [1280:1285]=biases
    cpak = nc.dram_tensor("cpak", [P, 1285], F32, kind="ExternalInput").ap()
    oq = nc.dram_tensor("oq", [CO, Q], F32, kind="ExternalOutput").ap()

    with tile.TileContext(nc) as tc:
        with (
            tc.tile_pool(name="consts", bufs=1) as consts,
            tc.tile_pool(name="big", bufs=1) as big,
            tc.tile_pool(name="tmp", bufs=6) as tmp,
        ):
            # ---- constants on the sync queue; x chunks spread across the
            # scalar/vector/gpsimd queues in need-order so the first
            # projection can start as early as possible ----
            cpak_sb = consts.tile([P, 1285], MM_DT, tag="cpak")
            nc.sync.dma_start(cpak_sb[:], cpak.bitcast(MM_DT))
            wtT = cpak_sb[:, 0:256].rearrange("p (o c) -> p o c", o=2)
            wpT = cpak_sb[:, 256:512].rearrange("p (o c) -> p o c", o=2)
            wgT = cpak_sb[:, 512:768].rearrange("p (o c) -> p o c", o=2)
            woT = cpak_sb[:, 768:1024].rearrange("p (o c) -> p o c", o=2)
            identity_r = cpak_sb[:, 1024:1152]
            ones_sb = cpak_sb[:, 1152:1280]
            bt_sb = cpak_sb[:, 1280:1281].bitcast(F32)
            bp_sb = cpak_sb[:, 1281:1282].bitcast(F32)
            bg_sb = cpak_sb[:, 1282:1283].bitcast(F32)
            bo_sb = cpak_sb[:, 1283:1285].bitcast(F32)

            x_sb = big.tile([P, 2, N], MM_DT, tag="x")
            xbr = xb.rearrange("(o p) n -> p o n", p=P).bitcast(MM_DT)
            # need-ordered x chunks; the startup critical path is cpak+x0:
            # serialize the next-needed chunks behind them on the sync queue
            # so they don't steal DMA bandwidth, and gate the late tail
            # chunks (gpsimd/SWDGE) behind the first theta evacuation via a
            # tiny copy so their transfers start only after the critical
            # window
            nc.scalar.dma_start(x_sb[:, :, 0:512], xbr[:, :, 0:512])
            nc.sync.dma_start(x_sb[:, :, 512:1024], xbr[:, :, 512:1024])
            nc.scalar.dma_start(x_sb[:, :, 1024:2048], xbr[:, :, 1024:2048])
            nc.sync.dma_start(x_sb[:, :, 2048:3072], xbr[:, :, 2048:3072])
            nc.scalar.dma_start(x_sb[:, :, 3072:4096], xbr[:, :, 3072:4096])

            # SBUF buffers shared across phases
            theta_sb = big.tile([P, Q], MM_DT, tag="theta")
            phi_sb = big.tile([P, N], MM_DT, tag="phi")
            g_sb = big.tile([P, N], MM_DT, tag="g")
            gT_sb = big.tile([P, N], MM_DT, tag="gT")  # kb-blocked g^T
            y_sb = big.tile([P, Q], MM_DT, tag="y")
            d_acc = big.tile([P, Q], MM_DT, tag="dacc")
            out_sb = big.tile([P, 2, Q], F32, tag="out")
            oqr = oq.rearrange("(o p) q -> p o q", p=P)

            def make_attn_group(ps_s, ps_y, ps_proj=None, work=None):
                def attn_group(gi, q0, qw, pending_out=None, final=False):
                    """One query group's attention.  scores/y on PE, exp on
                    Act, softmax denominator accumulated on DVE (d_acc +=
                    exp tile).  Deferred projection pieces (`work`, group 0)
                    and the previous group's output projection (pending_out,
                    one piece per kb) are interleaved to fill engine gaps.
                    Returns a list of output-projection piece closures."""
                    qsl = slice(q0, q0 + qw)
                    nh = qw // 512
                    with nc.named_scope(f"attn{gi}"):
                        y_ps = ps_y.tile([P, qw], F32, tag="y", name=f"y_ps{gi}")

                        def scores(kb):
                            s_ps = ps_s.tile(
                                [P, qw], F32, tag="s", name=f"s{gi}_{kb}"
                            )
                            for h in range(nh):
                                nc.tensor.matmul(
                                    s_ps[:, h * 512 : (h + 1) * 512],
                                    phi_sb[:, kb * P : (kb + 1) * P],
                                    theta_sb[:, q0 + h * 512 : q0 + (h + 1) * 512],
                                    start=True, stop=True,
                                )
                            return s_ps

                        s_cur = scores(0)
                        for kb in range(NKB):
                            at = tmp.tile(
                                [P, qw], MM_DT, tag="attn", name=f"at{gi}_{kb}"
                            )
                            if final and kb == NKB - 1 and nh > 1:
                                # last exp of the kernel: split per 512 so
                                # the tail epilogue starts sooner
                                for h in range(nh):
                                    hsl = slice(h * 512, (h + 1) * 512)
                                    nc.scalar.activation(
                                        out=at[:, hsl], in_=s_cur[:, hsl],
                                        func=AF.Exp,
                                    )
                            else:
                                nc.scalar.activation(
                                    out=at[:], in_=s_cur[:], func=AF.Exp
                                )
                            if kb + 1 < NKB:
                                # feed the PE the next scores before y(kb) so
                                # it is not idle while Act runs exp(kb)
                                s_cur = scores(kb + 1)
                            # one deferred projection piece per kb (group 0)
                            if work:
                                fn, arg = work.pop(0)
                                fn(*arg)
                            first, last = kb == 0, kb == NKB - 1
                            for h in range(nh):
                                hsl = slice(h * 512, (h + 1) * 512)
                                nc.tensor.matmul(
                                    y_ps[:, hsl], gT_sb[:, kb * P : (kb + 1) * P],
                                    at[:, hsl], start=first, stop=last,
                                )
                            # softmax denominator on DVE
                            if kb == 0:
                                nc.vector.tensor_copy(
                                    out=d_acc[:, qsl], in_=at[:]
                                )
                            else:
                                nc.vector.tensor_tensor(
                                    out=d_acc[:, qsl], in0=d_acc[:, qsl],
                                    in1=at[:], op=OP.add,
                                )
                            # previous group's output projection, one piece
                            # per kb starting at kb=2 (spreads the PSUM-slot
                            # churn instead of a serializing burst)
                            if pending_out and kb >= 2:
                                pending_out.pop(0)(ps_s)
                        while pending_out:
                            pending_out.pop(0)(ps_s)
                        # evacuate y before this group's PSUM scope can be
                        # torn down (the deferred pieces run in the next
                        # group's scope); Act is between exp streams here
                        for h in range(nh):
                            hsl = slice(h * 512, (h + 1) * 512)
                            qhsl = slice(q0 + h * 512, q0 + (h + 1) * 512)
                            nc.scalar.activation(
                                out=y_sb[:, qhsl], in_=y_ps[:, hsl], func=AF.Copy
                            )

                    pieces = []
                    rd = tmp.tile([P, qw], F32, tag="rd", name=f"rd{gi}")

                    def epi_h(pool, h):
                        # per-512 epilogue: denominator broadcast
                        # (ones-matmul) and reciprocal
                        hsl = slice(h * 512, (h + 1) * 512)
                        qhsl = slice(q0 + h * 512, q0 + (h + 1) * 512)
                        with nc.named_scope(f"epi{gi}"):
                            d_bc = pool.tile(
                                [P, 512], F32, tag="s", name=f"dbc{gi}{h}"
                            )
                            nc.tensor.matmul(
                                d_bc[:], ones_sb, d_acc[:, qhsl],
                                start=True, stop=True,
                            )
                            nc.vector.reciprocal_approx_fast(
                                out=rd[:, hsl], in_=d_bc[:],
                            )

                    def out_piece(pool, blk, h):
                        hsl = slice(h * 512, (h + 1) * 512)
                        qhsl = slice(q0 + h * 512, q0 + (h + 1) * 512)
                        with nc.named_scope(f"outp{gi}"):
                            po = pool.tile(
                                [P, 512], F32, tag="s", name=f"po{gi}{blk}{h}"
                            )
                            nc.tensor.matmul(
                                po[:], woT[:, blk, :], y_sb[:, qhsl],
                                start=True, stop=True,
                            )
                            # out = (po + b_out_eff) * rd in one DVE pass
                            nc.vector.scalar_tensor_tensor(
                                out=out_sb[:, blk, qhsl], in0=po[:],
                                scalar=bo_sb[:, blk : blk + 1], in1=rd[:, hsl],
                                op0=OP.add, op1=OP.mult,
                            )
                            nc.sync.dma_start(
                                oqr[:, blk, qhsl], out_sb[:, blk, qhsl]
                            )

                    # h-major so each half's chain drains independently
                    for h in range(nh):
                        pieces.append(lambda pool, h=h: epi_h(pool, h))
                        for blk in range(2):
                            pieces.append(
                                lambda pool, blk=blk, h=h: out_piece(pool, blk, h)
                            )
                    if final:
                        for p in pieces:
                            p(ps_s)
                        return []
                    return pieces

                return attn_group

            with (
                tc.tile_pool(name="ps_sA", bufs=2, space="PSUM") as ps_sA,
                tc.tile_pool(name="ps_yA", bufs=1, space="PSUM") as ps_yA,
                tc.tile_pool(name="ps_proj", bufs=2, space="PSUM") as ps_proj,
            ):
                # ---- projection emitters ----------------------------------
                def proj(which, j, on_act=False):
                    wT, bias, dst = {
                        "t": (wtT, bt_sb, theta_sb),
                        "p": (wpT, bp_sb, phi_sb),
                        "g": (wgT, bg_sb, g_sb),
                    }[which]
                    sl = slice(j * 512, (j + 1) * 512)
                    pp = ps_proj.tile([P, 512], F32, tag="pp", name=f"pp{which}{j}")
                    nc.tensor.matmul(
                        pp[:], wT[:, 0, :], x_sb[:, 0, sl], start=True, stop=False
                    )
                    nc.tensor.matmul(
                        pp[:], wT[:, 1, :], x_sb[:, 1, sl], start=False, stop=True
                    )
                    if on_act:
                        # only used in the prologue, before the exp stream
                        nc.scalar.activation(
                            out=dst[:, sl], in_=pp[:], func=AF.Identity,
                            bias=bias,
                        )
                    else:
                        nc.vector.tensor_scalar(
                            out=dst[:, sl], in0=pp[:],
                            scalar1=bias, scalar2=None, op0=OP.add,
                        )

                def gtq(j, on_act=False):
                    # transpose g columns 512j..512j+512 (4 key blocks) into
                    # bitcast slices of one projection-ring PSUM tile, then
                    # one copy into gT.  (f32r transposes run at 1.5
                    # cycles/row; direct gT-from-x matmuls would be 128-wide
                    # f32r = quarter rate, slower.)
                    sl = slice(j * 512, (j + 1) * 512)
                    pq = ps_proj.tile([P, 512], F32, tag="pp", name=f"pq{j}")
                    for k in range(4):
                        ksl = slice(j * 512 + k * P, j * 512 + (k + 1) * P)
                        nc.tensor.transpose(
                            pq[:, k * P : (k + 1) * P].bitcast(MM_DT),
                            g_sb[:, ksl], identity_r,
                        )
                    if on_act:
                        nc.scalar.activation(
                            out=gT_sb[:, sl], in_=pq[:].bitcast(MM_DT),
                            func=AF.Copy,
                        )
                    else:
                        nc.vector.tensor_copy(
                            out=gT_sb[:, sl], in_=pq[:].bitcast(MM_DT)
                        )

                # deferred pieces, one per kb of group 0.  phi j feeds
                # scores kb=4j (emitted one kb early); gT quad j feeds y
                # kb=4j; theta j2/j3 feed group 1.
                work = [
                    (proj, ("p", 1)), (proj, ("g", 1)), (proj, ("t", 2)),
                    (gtq, (1,)),
                ]
                for j in range(2, 8):
                    work += [(proj, ("p", j)), (proj, ("g", j)), (gtq, (j,))]
                work += [(proj, ("t", 3))]
                # deadline check: piece k emits at kb=k.  quad j at 3j <=
                # its deadline y(4j) (quad1 at 3 <= 4); phi j at 3j-2 <=
                # 4j-1.  ok.

                # prologue: minimum for scores/y at kb=0..3, evacuated on
                # the still-idle Act engine
                proj("t", 0, on_act=True)
                proj("p", 0, on_act=True)
                proj("g", 0, on_act=True)
                gtq(0, on_act=True)
                proj("t", 1, on_act=True)

                grpA = make_attn_group(ps_sA, ps_yA, ps_proj, work)
                out0 = grpA(0, 0, 1024)

            with (
                tc.tile_pool(name="ps_sB", bufs=3, space="PSUM") as ps_sB,
                tc.tile_pool(name="ps_yB", bufs=1, space="PSUM") as ps_yB,
            ):
                grpB = make_attn_group(ps_sB, ps_yB)
                grpB(1, 1024, 1024, pending_out=out0, final=True)

    nc.compile()
    return nc


IDENT = np.eye(P, dtype=np.float32)

_NC_CACHE = None
LAST_EXEC_TIME_NS = None
LAST_TRACE = None
LAST_RESULTS = None


def _get_nc():
    global _NC_CACHE
    if _NC_CACHE is None:
        _NC_CACHE = build()
    return _NC_CACHE


def kernel(**inputs):
    x = np.ascontiguousarray(np.asarray(inputs["x"], dtype=np.float32))
    assert x.shape == (B, CI, T, H, W), x.shape
    xf = x.reshape(B, CI, N)
    w = {
        k: np.ascontiguousarray(np.asarray(inputs[k], dtype=np.float32))
        for k in (
            "w_theta", "b_theta", "w_phi", "b_phi", "w_g", "b_g", "w_out", "b_out"
        )
    }

    def proj_t(wm):
        # [p, o*128+c] = wm[c, o*128+p]
        return wm.T.reshape(2, P, P).transpose(1, 0, 2).reshape(P, 2 * P)

    woT_h = w["w_out"].reshape(2, P, CINT).transpose(2, 0, 1).reshape(P, 2 * P)
    CPAK = np.ascontiguousarray(
        np.concatenate(
            [
                proj_t(w["w_theta"]), proj_t(w["w_phi"]), proj_t(w["w_g"]),
                woT_h, IDENT, np.ones((P, P), np.float32),
                np.stack(
                    [
                        w["b_theta"], w["b_phi"], w["b_g"],
                        w["b_out"][:P], w["b_out"][P:],
                    ],
                    axis=1,
                ),
            ],
            axis=1,
        )
    )
    in_maps = []
    for core in range(8):
        b, h = core // 2, core % 2
        if h == 0:
            xcore = xf[b]
        else:
            xcore = np.ascontiguousarray(
                np.concatenate([xf[b][:, Q:], xf[b][:, :Q]], axis=1)
            )
        in_maps.append(
            {"xb": xcore, "cpak": CPAK}
        )

    nc = _get_nc()
    res = run_bass_kernel_spmd(nc, in_maps, core_ids=list(range(8)))
    global LAST_EXEC_TIME_NS, LAST_TRACE, LAST_RESULTS
    LAST_EXEC_TIME_NS = res.exec_time_ns
    LAST_TRACE = res.instructions_and_trace[1] if res.instructions_and_trace else None
    LAST_RESULTS = res

    out = np.empty((B, CO, N), np.float32)
    for core in range(8):
        b, h = core // 2, core % 2
        out[b][:, h * Q : (h + 1) * Q] = res.results[core]["oq"]
    return out.reshape(B, CO, T, H, W)
